# revision 28
# baseline (speedup 1.0000x reference)
"""Distributed causal-attention kernel for TRN2 (8 NeuronCores).

Module: qkv = x@w_attn+b; q,k l2-normalized per head; scaled (8.0) causal
softmax attention; out = (attn@v reassembled)@w_proj + b_proj.
Shapes: x [2,2048,1024], 16 heads x 64 dim.

Sharding: pure tensor-parallel over heads (2 heads/core).  Each core
computes qkv for its heads over the full batch*seq, runs attention, then two
8-core AllToAlls (one per head, pipelined against compute) redistribute the
per-head outputs to row-shards so each core applies the full output
projection to its 512 rows.

v3 structure (the chip throttles the PE clock to 1.2 GHz after ~70us of
8-core activity, so total PE cycles and engine overlap dominate):
 - phase B: qkv matmuls with a per-tile norm pipeline (recip on DVE +
   Sqrt on ACT -- one table set each for the whole phase) and the q/k
   transposes emitted per tile so they hide under the next tile's matmuls
   instead of forming a serial tail
 - phase C: scores are computed transposed [k, q]; the exp'd tile is
   directly the AV matmul's stationary operand (the 65-column AV streams
   pipeline at N-cycle spacing with FWL hiding the loads, so many small
   AV matmuls beat fewer large ones); the softmax denominator comes from
   a ones column appended to v; the divide is a per-partition scalar
   multiply on the [q, hd] AV output
 - o is transposed on-device pre-A2A so the collective payload is oT and
   the receive side is a single plain DMA into the projection layout
 - phase D: both heads' oT shards stacked into 128 partitions so the
   projection contracts K=128 per matmul (and runs after both A2As)
"""
import sys

if '/opt/trn_rl_repo' not in sys.path:
    sys.path.insert(0, '/opt/trn_rl_repo')

import numpy as np
import ml_dtypes

import concourse.bass as bass
import concourse.mybir as mybir
from concourse import bacc, tile
from concourse.bass import ts, ds
from concourse.bass_utils import run_bass_kernel_spmd
from concourse.masks import make_identity

B, S, D, H = 2, 2048, 1024, 16
HD = D // H                 # 64
NCORES = 8
HPC = H // NCORES           # 2 heads per core
SEQT = 128
NT = (B * S) // SEQT        # 32 seq tiles (batch-major)
TPB = S // SEQT             # 16 tiles per batch
QSPAN = 512
NSPAN = S // QSPAN          # 4 q-spans per batch
ROWS = (B * S) // NCORES    # 512 output rows per core
KC = D // 128               # 8 contraction chunks
W3 = 3 * HPC * HD           # 384 qkv columns per core
BF = mybir.dt.bfloat16
F32 = mybir.dt.float32
AF = mybir.ActivationFunctionType
MUL = mybir.AluOpType.mult


def build(dbg=False, with_bias=False):
    nc = bacc.Bacc("TRN2", target_bir_lowering=False, debug=False,
                   num_devices=NCORES)
    xt = nc.dram_tensor("xt", [D, B * S], BF, kind="ExternalInput")
    wq = nc.dram_tensor("wq", [D, W3], BF, kind="ExternalInput")
    ba = nc.dram_tensor("ba", [1, W3], BF, kind="ExternalInput")
    wp = nc.dram_tensor("wp", [D, D], BF, kind="ExternalInput")
    bp = nc.dram_tensor("bp", [1, D], BF, kind="ExternalInput")
    out = nc.dram_tensor("out", [ROWS, D], F32, kind="ExternalOutput")

    with tile.TileContext(nc) as tc:
        with tc.tile_pool(name="persist", bufs=1) as pp, \
             tc.tile_pool(name="dram", bufs=1, space="DRAM") as dram, \
             tc.tile_pool(name="work", bufs=4) as work, \
             tc.tile_pool(name="epool", bufs=12) as epool:

            # ---- persistent SBUF ----
            xt_sb = pp.tile([128, KC, B * S], BF, name="xt_sb")
            wq_sb = pp.tile([128, KC, W3], BF, name="wq_sb")
            wp_sb = pp.tile([128, KC, D], BF, name="wp_sb")
            ba_sb = pp.tile([1, W3], BF, name="ba_sb")
            bp_sb = pp.tile([1, D], BF, name="bp_sb")
            ones_sb = pp.tile([1, 128], BF, name="ones_sb")
            tri = pp.tile([128, 128], BF, name="tri")
            # q,k working copies (normalized in place) + per-tile norm stats
            qk_all = pp.tile([128, NT, 2 * HPC * HD], BF, name="qk_all")
            n2_all = pp.tile([128, NT, 2 * HPC], F32, name="n2_all")
            ri_all = pp.tile([128, NT, 2 * HPC], F32, name="ri_all")
            rn_all = pp.tile([128, NT, 2 * HPC], F32, name="rn_all")
            # qT/kT per batch: head0 rows 0:64, head1 rows 64:128
            QT = [pp.tile([128, S], BF, name=f"qt{b}") for b in range(B)]
            KT = [pp.tile([128, S], BF, name=f"kt{b}") for b in range(B)]
            # v in [seq, hd] layout; per head: cols 0:64 = v, col 64 = ones
            # (softmax denominator via the AV matmul)
            v_sb = pp.tile([128, NT, 2, 72], BF, name="v_sb")
            # both heads stacked: rows 64h:64h+64 = head h  ->  K=128 proj
            ocT = pp.tile([128, NCORES, ROWS], BF, name="ocT")

            # A2A payloads are oT, one collective per (head, batch) quarter
            # so the last one is only 256KB.  Tokens are interleaved across
            # cores (core p owns global seq-tiles {p, p+8, p+16, p+24}) so
            # every quarter carries data for every destination core.
            a2a_in = [[dram.tile([64 * NCORES, 256], BF,
                                 name=f"a2a_in{h}_{b}") for b in range(B)]
                      for h in range(HPC)]
            a2a_out = [[dram.tile([64 * NCORES, 256], BF,
                                  name=f"a2a_out{h}_{b}") for b in range(B)]
                       for h in range(HPC)]

            # ---- constants (ordered so phase B can start early: the t=0
            # matmul over chunk kc needs only wq[kc] + xt[kc, 0:128]) ----
            for kc in range(KC):
                nc.sync.dma_start(wq_sb[:, kc, :], wq[ts(kc, 128), :])
                nc.sync.dma_start(xt_sb[:, kc, ds(0, 128)],
                                  xt[ts(kc, 128), ds(0, 128)])
            nc.gpsimd.memset(ones_sb[:], 1.0)
            nc.gpsimd.memset(v_sb[:], 0.0)
            nc.gpsimd.memset(
                v_sb[:].rearrange("p a b c -> p (a b) c")[:, :, HD:HD + 1],
                1.0)
            # tri[k, q] = 1 where q >= k (valid causal), else 0
            nc.gpsimd.memset(tri[:], 1.0)
            nc.gpsimd.affine_select(
                out=tri[:], in_=tri[:], compare_op=mybir.AluOpType.is_ge,
                fill=0.0, base=0, pattern=[[1, 128]], channel_multiplier=-1)
            for kc in range(KC):
                nc.sync.dma_start(xt_sb[:, kc, ds(128, 896)],
                                  xt[ts(kc, 128), ds(128, 896)])
            for tq in range(1, 4):
                for kc in range(KC):
                    nc.sync.dma_start(xt_sb[:, kc, ds(1024 * tq, 1024)],
                                      xt[ts(kc, 128), ds(1024 * tq, 1024)])
            for kc in range(KC):
                nc.sync.dma_start(wp_sb[:, kc, :], wp[ts(kc, 128), :])
            nc.sync.dma_start(ba_sb[:], ba[:])
            nc.sync.dma_start(bp_sb[:], bp[:])

            # ---- phase B: qkv matmuls + per-tile normalize/transpose ----
            ps_qkv_ctx = tc.tile_pool(name="ps_qkv", bufs=4, space="PSUM")
            ps_qkv = ps_qkv_ctx.__enter__()
            for t in range(NT):
                b_, tt = divmod(t, TPB)
                ps = ps_qkv.tile([128, W3], F32, tag="ps", name=f"ps{t}")
                for kc in range(KC):
                    nc.tensor.matmul(ps[:], lhsT=xt_sb[:, kc, ts(t, 128)],
                                     rhs=wq_sb[:, kc, :], start=(kc == 0),
                                     stop=(not with_bias and kc == KC - 1))
                if with_bias:
                    nc.tensor.matmul(ps[:], lhsT=ones_sb[:, 0:128],
                                     rhs=ba_sb[:], start=False, stop=True)
                nc.scalar.copy(qk_all[:, t, :], ps[:, 0:256])
                nc.scalar.copy(
                    v_sb[:, t, :, 0:HD],
                    ps[:, 256:384].rearrange("p (h e) -> p h e", e=HD))
                sq = work.tile([128, 2 * HPC * HD], BF, tag="sq",
                               name=f"sq{t}")
                nc.vector.tensor_mul(sq[:], qk_all[:, t, :], qk_all[:, t, :])
                nc.vector.reduce_sum(
                    n2_all[:, t, :], sq[:].rearrange("p (g e) -> p g e", e=HD),
                    axis=mybir.AxisListType.X)
                # rn = sqrt(8/n2): recip on DVE (accuracy), Sqrt on ACT --
                # single table set kept loaded for the whole phase
                nc.vector.reciprocal(ri_all[:, t, :], n2_all[:, t, :])
                nc.scalar.activation(rn_all[:, t, :], ri_all[:, t, :],
                                     AF.Sqrt, scale=8.0)
                nc.vector.tensor_tensor(
                    qk_all[:, t, :].rearrange("p (g e) -> p g e", e=HD),
                    qk_all[:, t, :].rearrange("p (g e) -> p g e", e=HD),
                    rn_all[:, t, :, None].broadcast_to([128, 4, HD]), op=MUL)
                # q/k transposes on the DMA XBAR (ACT-triggered): frees the
                # PE and DVE entirely; ~112ns of DMA-engine time each
                nc.scalar.dma_start_transpose(
                    QT[b_][:, ts(tt, 128)], qk_all[:, t, 0:128])
                nc.scalar.dma_start_transpose(
                    KT[b_][:, ts(tt, 128)], qk_all[:, t, 128:256])

            ps_qkv_ctx.__exit__(None, None, None)
            psB_ctx = tc.tile_pool(name="psB", bufs=4, space="PSUM")
            psB = psB_ctx.__enter__()
            psC_ctx = tc.tile_pool(name="psC", bufs=4, space="PSUM")
            psC = psC_ctx.__enter__()

            # ---- phase C: attention (head-major for A2A pipelining) ----
            for h in range(HPC):
                for b_ in range(B):
                    for j in range(NSPAN):
                        oaccs = [work.tile([128, 2, HD], BF, tag="oacc",
                                           bufs=6,
                                           name=f"oacc{h}_{b_}_{j}_{cp}")
                                 for cp in range(2)]
                        nk = 4 * j + 4
                        avs = [psC.tile([128, HD + 1], F32, tag="av",
                                        name=f"av{b_}_{j}_{h}_{c}")
                               for c in range(4)]
                        for i in range(nk):
                            d = i - 4 * j
                            c0 = max(d, 0)
                            sps = psB.tile([128, QSPAN], F32, tag="s",
                                           name=f"s{b_}_{j}_{h}_{i}")
                            # stream only the causally-needed q columns
                            nc.tensor.matmul(
                                sps[:, 128 * c0:],
                                lhsT=KT[b_][64 * h:64 * h + 64, ts(i, 128)],
                                rhs=QT[b_][64 * h:64 * h + 64,
                                           ds(j * QSPAN + 128 * c0,
                                              QSPAN - 128 * c0)],
                                start=True, stop=True)
                            e = epool.tile([128, QSPAN], BF, tag="e",
                                           name=f"e{b_}_{j}_{h}_{i}")
                            nc.scalar.activation(e[:, 128 * c0:],
                                                 sps[:, 128 * c0:], AF.Exp)
                            if d >= 0:
                                nc.vector.tensor_tensor(
                                    e[:, 128 * d:128 * (d + 1)],
                                    e[:, 128 * d:128 * (d + 1)], tri[:],
                                    op=MUL)
                            for c in range(c0, 4):
                                nc.tensor.matmul(
                                    avs[c][:],
                                    lhsT=e[:, ts(c, 128)],
                                    rhs=v_sb[:, b_ * TPB + i, h, 0:HD + 1],
                                    start=(i == 0), stop=(i == 4 * j + c))
                        # stage-batched epilogue: independent per-subtile
                        # ops emitted stage-major so they pipeline instead of
                        # forming four serial 5-stage chains
                        rd = work.tile([128, 4], F32, tag="rd",
                                       name=f"rd{b_}_{j}_{h}")
                        for c in range(4):
                            nc.vector.reciprocal(rd[:, c:c + 1],
                                                 avs[c][:, HD:HD + 1])
                        for c in range(4):
                            nc.vector.tensor_scalar_mul(
                                oaccs[c // 2][:, c % 2, :],
                                avs[c][:, 0:HD], rd[:, c:c + 1])
                        for cp in range(2):
                            # transpose a [128, 2*64] subtile pair on the
                            # DMA XBAR; result rows 0:64 / 64:128 are the
                            # two seq-tiles' oT, which land contiguously in
                            # the A2A payload (tile u -> core u%8, slot u//8)
                            ot = work.tile([128, 128], BF, tag="ot", bufs=8,
                                           name=f"ots{h}_{b_}_{j}_{cp}")
                            nc.sync.dma_start_transpose(
                                ot[:],
                                oaccs[cp][:].rearrange("p a b -> p (a b)"))
                            u0 = 4 * j + 2 * cp
                            nc.gpsimd.dma_start(
                                a2a_in[h][b_][ds(64 * (u0 % 8), 128),
                                              ds((u0 // 8) * 128, 128)],
                                ot[:])
                    # quarter collective: fires as soon as this (head,
                    # batch) is out, overlapping the remaining attention
                    nc.gpsimd.collective_compute(
                        "AllToAll", mybir.AluOpType.bypass,
                        replica_groups=[list(range(NCORES))],
                        ins=[a2a_in[h][b_][:].opt()],
                        outs=[a2a_out[h][b_][:].opt()])
                    nc.sync.dma_start(
                        ocT[64 * h:64 * h + 64, :, ds(256 * b_, 256)],
                        a2a_out[h][b_][:].rearrange("(p c) w -> c p w",
                                                    c=64))

            # ---- phase D: projection, both heads stacked so each matmul
            # contracts K=128 ----
            for rt in range(ROWS // 128):
                for half in range(2):
                    yps = psB.tile([128, QSPAN], F32, tag="s",
                                   name=f"y_{rt}_{half}")
                    for p in range(NCORES):
                        nc.tensor.matmul(
                            yps[:], lhsT=ocT[:, p, ts(rt, 128)],
                            rhs=wp_sb[:, p, ds(half * 512, 512)],
                            start=(p == 0),
                            stop=(not with_bias and p == NCORES - 1))
                    if with_bias:
                        nc.tensor.matmul(yps[:], lhsT=ones_sb[:, 0:128],
                                         rhs=bp_sb[:, ds(half * 512, 512)],
                                         start=False, stop=True)
                    ysb = work.tile([128, 512], F32, tag="y", bufs=8,
                                    name=f"ysb{rt}_{half}")
                    nc.vector.tensor_copy(ysb[:], yps[:])
                    nc.sync.dma_start(
                        out[ts(rt, 128), ds(half * 512, 512)], ysb[:])

            psC_ctx.__exit__(None, None, None)
            psB_ctx.__exit__(None, None, None)

    nc.compile()
    return nc


_NC = None


def _get_nc(with_bias=False):
    global _NC
    if _NC is None or _NC[1] != with_bias:
        _NC = (build(with_bias=with_bias), with_bias)
    return _NC[0]


def make_in_maps(x, w_attn, b_attn, w_proj, b_proj):
    bf = ml_dtypes.bfloat16
    xt = np.ascontiguousarray(x.reshape(B * S, D).T).astype(bf)
    wp_ = np.ascontiguousarray(w_proj).astype(bf)
    bp_ = b_proj.reshape(1, D).astype(bf)
    in_maps = []
    for c in range(NCORES):
        sl = slice(128 * c, 128 * c + 128)
        wq_ = np.ascontiguousarray(np.concatenate(
            [w_attn[:, sl], w_attn[:, 1024:2048][:, sl],
             w_attn[:, 2048:3072][:, sl]], axis=1)).astype(bf)
        ba_ = np.concatenate(
            [b_attn[sl], b_attn[1024:2048][sl],
             b_attn[2048:3072][sl]]).reshape(1, W3).astype(bf)
        in_maps.append({"xt": xt, "wq": wq_, "ba": ba_, "wp": wp_, "bp": bp_})
    return in_maps


def gather_out(results):
    # core p owns global seq-tiles {p, p+8, p+16, p+24} (128 rows each)
    out = np.empty((B * S, D), np.float32)
    for g in range(NT):
        out[128 * g:128 * (g + 1), :] = \
            results[g % 8]["out"][128 * (g // 8):128 * (g // 8) + 128]
    return out.reshape(B, S, D)


def kernel(x, w_attn, b_attn, w_proj, b_proj):
    with_bias = bool(np.any(b_attn) or np.any(b_proj))
    nc = _get_nc(with_bias=with_bias)
    in_maps = make_in_maps(np.asarray(x, np.float32), np.asarray(w_attn, np.float32),
                           np.asarray(b_attn, np.float32),
                           np.asarray(w_proj, np.float32),
                           np.asarray(b_proj, np.float32))
    res = run_bass_kernel_spmd(nc, in_maps, core_ids=list(range(NCORES)))
    return gather_out(res.results)


# revision 35
# speedup vs baseline: 1.5632x; 1.5632x over previous
"""Distributed causal-attention kernel for TRN2 (8 NeuronCores).

Module: qkv = x@w_attn+b; q,k l2-normalized per head; scaled (8.0) causal
softmax attention; out = (attn@v reassembled)@w_proj + b_proj.
Shapes: x [2,2048,1024], 16 heads x 64 dim.

Sharding: pure tensor-parallel over heads (2 heads/core).  Each core
computes qkv for its heads over the full batch*seq, runs attention, then two
8-core AllToAlls (one per head, pipelined against compute) redistribute the
per-head outputs to row-shards so each core applies the full output
projection to its 512 rows.

v3 structure (the chip throttles the PE clock to 1.2 GHz after ~70us of
8-core activity, so total PE cycles and engine overlap dominate):
 - phase B: qkv matmuls with a per-tile norm pipeline (recip on DVE +
   Sqrt on ACT -- one table set each for the whole phase) and the q/k
   transposes emitted per tile so they hide under the next tile's matmuls
   instead of forming a serial tail
 - phase C: scores are computed transposed [k, q]; the exp'd tile is
   directly the AV matmul's stationary operand (the 65-column AV streams
   pipeline at N-cycle spacing with FWL hiding the loads, so many small
   AV matmuls beat fewer large ones); the softmax denominator comes from
   a ones column appended to v; the divide is a per-partition scalar
   multiply on the [q, hd] AV output
 - o is transposed on-device pre-A2A so the collective payload is oT and
   the receive side is a single plain DMA into the projection layout
 - phase D: both heads' oT shards stacked into 128 partitions so the
   projection contracts K=128 per matmul (and runs after both A2As)
"""
import sys

if '/opt/trn_rl_repo' not in sys.path:
    sys.path.insert(0, '/opt/trn_rl_repo')

import numpy as np
import ml_dtypes

import concourse.bass as bass
import concourse.mybir as mybir
from concourse import bacc, tile
from concourse.bass import ts, ds
from concourse.bass_utils import run_bass_kernel_spmd
from concourse.masks import make_identity

B, S, D, H = 2, 2048, 1024, 16
HD = D // H                 # 64
NCORES = 8
HPC = H // NCORES           # 2 heads per core
SEQT = 128
NT = (B * S) // SEQT        # 32 seq tiles (batch-major)
TPB = S // SEQT             # 16 tiles per batch
QSPAN = 512
NSPAN = S // QSPAN          # 4 q-spans per batch
ROWS = (B * S) // NCORES    # 512 output rows per core
KC = D // 128               # 8 contraction chunks
W3 = 3 * HPC * HD           # 384 qkv columns per core
BF = mybir.dt.bfloat16
F32 = mybir.dt.float32
AF = mybir.ActivationFunctionType
MUL = mybir.AluOpType.mult


def build(dbg=False, with_bias=False):
    nc = bacc.Bacc("TRN2", target_bir_lowering=False, debug=False,
                   num_devices=NCORES)
    xt = nc.dram_tensor("xt", [D, B * S], BF, kind="ExternalInput")
    wq = nc.dram_tensor("wq", [D, W3], BF, kind="ExternalInput")
    ba = nc.dram_tensor("ba", [1, W3], BF, kind="ExternalInput")
    wp = nc.dram_tensor("wp", [D, D], BF, kind="ExternalInput")
    bp = nc.dram_tensor("bp", [1, D], BF, kind="ExternalInput")
    out = nc.dram_tensor("out", [ROWS, D], F32, kind="ExternalOutput")

    with tile.TileContext(nc) as tc:
        with tc.tile_pool(name="persist", bufs=1) as pp, \
             tc.tile_pool(name="dram", bufs=1, space="DRAM") as dram, \
             tc.tile_pool(name="work", bufs=4) as work, \
             tc.tile_pool(name="epool", bufs=12) as epool:

            # ---- persistent SBUF ----
            xt_sb = pp.tile([128, KC, B * S], BF, name="xt_sb")
            wq_sb = pp.tile([128, KC, W3], BF, name="wq_sb")
            wp_sb = pp.tile([128, KC, D], BF, name="wp_sb")
            ba_sb = pp.tile([1, W3], BF, name="ba_sb")
            bp_sb = pp.tile([1, D], BF, name="bp_sb")
            ones_sb = pp.tile([1, 128], BF, name="ones_sb")
            ident = pp.tile([128, 128], BF, name="ident")
            tri = pp.tile([128, 128], BF, name="tri")
            # q,k working copies (normalized in place) + per-tile norm stats
            qk_all = pp.tile([128, NT, 2 * HPC * HD], BF, name="qk_all")
            n2_all = pp.tile([128, NT, 2 * HPC], F32, name="n2_all")
            ri_all = pp.tile([128, NT, 2 * HPC], F32, name="ri_all")
            rn_all = pp.tile([128, NT, 2 * HPC], F32, name="rn_all")
            # qT/kT per batch: head0 rows 0:64, head1 rows 64:128
            QT = [pp.tile([128, S], BF, name=f"qt{b}") for b in range(B)]
            KT = [pp.tile([128, S], BF, name=f"kt{b}") for b in range(B)]
            # v in [seq, hd] layout; per head: cols 0:64 = v, col 64 = ones
            # (softmax denominator via the AV matmul)
            v_sb = pp.tile([128, NT, 2, 72], BF, name="v_sb")
            # both heads stacked: rows 64h:64h+64 = head h  ->  K=128 proj;
            # split per batch so the b0 half of the projection can start
            # as soon as the two b0 collectives land
            ocT = [pp.tile([128, NCORES, 256], BF, name=f"ocT{b}")
                   for b in range(B)]

            # A2A payloads are oT, one collective per (head, batch) quarter
            # so the last one is only 256KB.  Tokens are interleaved across
            # cores (core p owns global seq-tiles {p, p+8, p+16, p+24}) so
            # every quarter carries data for every destination core.
            a2a_in = [[dram.tile([64 * NCORES, 256], BF,
                                 name=f"a2a_in{h}_{b}") for b in range(B)]
                      for h in range(HPC)]
            a2a_out = [[dram.tile([64 * NCORES, 256], BF,
                                  name=f"a2a_out{h}_{b}") for b in range(B)]
                       for h in range(HPC)]

            # ---- constants (ordered so phase B can start early: the t=0
            # matmul over chunk kc needs only wq[kc] + xt[kc, 0:128]) ----
            for kc in range(KC):
                nc.sync.dma_start(wq_sb[:, kc, :], wq[ts(kc, 128), :])
                nc.sync.dma_start(xt_sb[:, kc, ds(0, 384)],
                                  xt[ts(kc, 128), ds(0, 384)])
            nc.gpsimd.memset(ones_sb[:], 1.0)
            nc.gpsimd.memset(v_sb[:], 0.0)
            nc.gpsimd.memset(
                v_sb[:].rearrange("p a b c -> p (a b) c")[:, :, HD:HD + 1],
                1.0)
            make_identity(nc, ident[:])
            # tri[k, q] = 1 where q >= k (valid causal), else 0
            nc.gpsimd.memset(tri[:], 1.0)
            nc.gpsimd.affine_select(
                out=tri[:], in_=tri[:], compare_op=mybir.AluOpType.is_ge,
                fill=0.0, base=0, pattern=[[1, 128]], channel_multiplier=-1)
            for kc in range(KC):
                nc.sync.dma_start(xt_sb[:, kc, ds(384, 640)],
                                  xt[ts(kc, 128), ds(384, 640)])
            for tq in range(1, 4):
                for kc in range(KC):
                    nc.sync.dma_start(xt_sb[:, kc, ds(1024 * tq, 1024)],
                                      xt[ts(kc, 128), ds(1024 * tq, 1024)])
            for kc in range(KC):
                nc.sync.dma_start(wp_sb[:, kc, :], wp[ts(kc, 128), :])
            nc.sync.dma_start(ba_sb[:], ba[:])
            nc.sync.dma_start(bp_sb[:], bp[:])

            # ---- phase B: qkv matmuls + per-tile normalize/transpose ----
            ps_qkv_ctx = tc.tile_pool(name="ps_qkv", bufs=4, space="PSUM")
            ps_qkv = ps_qkv_ctx.__enter__()
            ps_tr_ctx = tc.tile_pool(name="ps_tr", bufs=2, space="PSUM")
            ps_tr = ps_tr_ctx.__enter__()
            for t in range(NT):
                b_, tt = divmod(t, TPB)
                ps = ps_qkv.tile([128, W3], F32, tag="ps", name=f"ps{t}")
                for kc in range(KC):
                    nc.tensor.matmul(ps[:], lhsT=xt_sb[:, kc, ts(t, 128)],
                                     rhs=wq_sb[:, kc, :], start=(kc == 0),
                                     stop=(not with_bias and kc == KC - 1))
                if with_bias:
                    nc.tensor.matmul(ps[:], lhsT=ones_sb[:, 0:128],
                                     rhs=ba_sb[:], start=False, stop=True)
                nc.scalar.copy(qk_all[:, t, :], ps[:, 0:256])
                nc.scalar.copy(
                    v_sb[:, t, :, 0:HD],
                    ps[:, 256:384].rearrange("p (h e) -> p h e", e=HD))
                sq = work.tile([128, 2 * HPC * HD], BF, tag="sq",
                               name=f"sq{t}")
                nc.vector.tensor_mul(sq[:], qk_all[:, t, :], qk_all[:, t, :])
                nc.vector.reduce_sum(
                    n2_all[:, t, :], sq[:].rearrange("p (g e) -> p g e", e=HD),
                    axis=mybir.AxisListType.X)
                # rn = sqrt(8/n2): recip on DVE (accuracy), Sqrt on ACT --
                # single table set kept loaded for the whole phase
                nc.vector.reciprocal(ri_all[:, t, :], n2_all[:, t, :])
                nc.scalar.activation(rn_all[:, t, :], ri_all[:, t, :],
                                     AF.Sqrt, scale=8.0)
                nc.vector.tensor_tensor(
                    qk_all[:, t, :].rearrange("p (g e) -> p g e", e=HD),
                    qk_all[:, t, :].rearrange("p (g e) -> p g e", e=HD),
                    rn_all[:, t, :, None].broadcast_to([128, 4, HD]), op=MUL)
                for src0, dst in ((0, QT[b_]), (128, KT[b_])):
                    trp = ps_tr.tile([128, 128], BF, tag="tr",
                                     name=f"tr{t}_{src0}")
                    nc.tensor.transpose(
                        trp[:], qk_all[:, t, src0:src0 + 128], ident[:])
                    nc.vector.tensor_copy(dst[:, ts(tt, 128)], trp[:])

            ps_tr_ctx.__exit__(None, None, None)
            ps_qkv_ctx.__exit__(None, None, None)
            ps_tr_ctx = tc.tile_pool(name="ps_tr2", bufs=2, space="PSUM")
            ps_tr = ps_tr_ctx.__enter__()
            psB_ctx = tc.tile_pool(name="psB", bufs=2, space="PSUM")
            psB = psB_ctx.__enter__()
            psC_ctx = tc.tile_pool(name="psC", bufs=4, space="PSUM")
            psC = psC_ctx.__enter__()

            # ---- phase C: attention (head-major for A2A pipelining) ----
            for h in range(HPC):
                for b_ in range(B):
                    for j in range(NSPAN):
                        oaccs = [work.tile([128, HD], BF, tag="oacc", bufs=10,
                                           name=f"oacc{h}_{b_}_{j}_{c}")
                                 for c in range(4)]
                        nk = 4 * j + 4
                        avs = [psC.tile([128, HD + 1], F32, tag="av",
                                        name=f"av{b_}_{j}_{h}_{c}")
                               for c in range(4)]
                        for i in range(nk):
                            d = i - 4 * j
                            c0 = max(d, 0)
                            sps = psB.tile([128, QSPAN], F32, tag="s",
                                           name=f"s{b_}_{j}_{h}_{i}")
                            # stream only the causally-needed q columns
                            nc.tensor.matmul(
                                sps[:, 128 * c0:],
                                lhsT=KT[b_][64 * h:64 * h + 64, ts(i, 128)],
                                rhs=QT[b_][64 * h:64 * h + 64,
                                           ds(j * QSPAN + 128 * c0,
                                              QSPAN - 128 * c0)],
                                start=True, stop=True)
                            e = epool.tile([128, QSPAN], BF, tag="e",
                                           name=f"e{b_}_{j}_{h}_{i}")
                            if h == 0 and b_ == 0 and j == 0 and i == 0:
                                # pin the first exp after the last B-phase
                                # Sqrt (write-write dep on e) so the ACT
                                # scheduler doesn't interleave them and
                                # thrash the activation table sets
                                nc.scalar.copy(e[0:1, 0:4],
                                               rn_all[0:1, NT - 1, :])
                            nc.scalar.activation(e[:, 128 * c0:],
                                                 sps[:, 128 * c0:], AF.Exp)
                            if d >= 0:
                                nc.vector.tensor_tensor(
                                    e[:, 128 * d:128 * (d + 1)],
                                    e[:, 128 * d:128 * (d + 1)], tri[:],
                                    op=MUL)
                            for c in range(c0, 4):
                                nc.tensor.matmul(
                                    avs[c][:],
                                    lhsT=e[:, ts(c, 128)],
                                    rhs=v_sb[:, b_ * TPB + i, h, 0:HD + 1],
                                    start=(i == 0), stop=(i == 4 * j + c))
                        # stage-batched epilogue: independent per-subtile
                        # ops emitted stage-major so they pipeline instead of
                        # forming four serial 5-stage chains
                        rd = work.tile([128, 4], F32, tag="rd",
                                       name=f"rd{b_}_{j}_{h}")
                        for c in range(4):
                            nc.vector.reciprocal(rd[:, c:c + 1],
                                                 avs[c][:, HD:HD + 1])
                        for c in range(4):
                            nc.vector.tensor_scalar_mul(
                                oaccs[c][:], avs[c][:, 0:HD], rd[:, c:c + 1])
                        trps = []
                        for c in range(4):
                            g = b_ * TPB + 4 * j + c
                            trp2 = ps_tr.tile([64, 128], BF, tag="tr",
                                              name=f"ot{h}_{g}")
                            nc.tensor.transpose(trp2[:], oaccs[c][:], ident[:])
                            trps.append(trp2)
                        ots = []
                        for c in range(4):
                            g = b_ * TPB + 4 * j + c
                            ot = work.tile([64, 128], BF, tag="ot", bufs=16,
                                           name=f"ots{h}_{g}")
                            nc.vector.tensor_copy(ot[:], trps[c][:])
                            ots.append(ot)
                        for c in range(4):
                            # seq-tile u of this batch goes to core u%8,
                            # slot u//8; gpsimd-triggered so these don't
                            # share queue credits with the sync queue
                            u = 4 * j + c
                            nc.gpsimd.dma_start(
                                a2a_in[h][b_][ts(u % 8, 64),
                                              ds((u // 8) * 128, 128)],
                                ots[c][:])
                    # quarter collective: fires as soon as this (head,
                    # batch) is out, overlapping the remaining attention
                    nc.gpsimd.collective_compute(
                        "AllToAll", mybir.AluOpType.bypass,
                        replica_groups=[list(range(NCORES))],
                        ins=[a2a_in[h][b_][:].opt()],
                        outs=[a2a_out[h][b_][:].opt()])
                    nc.sync.dma_start(
                        ocT[b_][64 * h:64 * h + 64, :, :],
                        a2a_out[h][b_][:].rearrange("(p c) w -> c p w",
                                                    c=64))

            # ---- phase D: projection, both heads stacked so each matmul
            # contracts K=128 ----
            for rt in range(ROWS // 128):
                for half in range(2):
                    yps = psB.tile([128, QSPAN], F32, tag="s",
                                   name=f"y_{rt}_{half}")
                    for p in range(NCORES):
                        nc.tensor.matmul(
                            yps[:], lhsT=ocT[rt // 2][:, p, ts(rt % 2, 128)],
                            rhs=wp_sb[:, p, ds(half * 512, 512)],
                            start=(p == 0),
                            stop=(not with_bias and p == NCORES - 1))
                    if with_bias:
                        nc.tensor.matmul(yps[:], lhsT=ones_sb[:, 0:128],
                                         rhs=bp_sb[:, ds(half * 512, 512)],
                                         start=False, stop=True)
                    ysb = work.tile([128, 512], F32, tag="y", bufs=8,
                                    name=f"ysb{rt}_{half}")
                    nc.vector.tensor_copy(ysb[:], yps[:])
                    nc.sync.dma_start(
                        out[ts(rt, 128), ds(half * 512, 512)], ysb[:])

            psC_ctx.__exit__(None, None, None)
            psB_ctx.__exit__(None, None, None)
            ps_tr_ctx.__exit__(None, None, None)

    nc.compile()
    return nc


_NC = None


def _get_nc(with_bias=False):
    global _NC
    if _NC is None or _NC[1] != with_bias:
        _NC = (build(with_bias=with_bias), with_bias)
    return _NC[0]


def make_in_maps(x, w_attn, b_attn, w_proj, b_proj):
    bf = ml_dtypes.bfloat16
    xt = np.ascontiguousarray(x.reshape(B * S, D).T).astype(bf)
    wp_ = np.ascontiguousarray(w_proj).astype(bf)
    bp_ = b_proj.reshape(1, D).astype(bf)
    in_maps = []
    for c in range(NCORES):
        sl = slice(128 * c, 128 * c + 128)
        wq_ = np.ascontiguousarray(np.concatenate(
            [w_attn[:, sl], w_attn[:, 1024:2048][:, sl],
             w_attn[:, 2048:3072][:, sl]], axis=1)).astype(bf)
        ba_ = np.concatenate(
            [b_attn[sl], b_attn[1024:2048][sl],
             b_attn[2048:3072][sl]]).reshape(1, W3).astype(bf)
        in_maps.append({"xt": xt, "wq": wq_, "ba": ba_, "wp": wp_, "bp": bp_})
    return in_maps


def gather_out(results):
    # core p owns global seq-tiles {p, p+8, p+16, p+24} (128 rows each)
    out = np.empty((B * S, D), np.float32)
    for g in range(NT):
        out[128 * g:128 * (g + 1), :] = \
            results[g % 8]["out"][128 * (g // 8):128 * (g // 8) + 128]
    return out.reshape(B, S, D)


def kernel(x, w_attn, b_attn, w_proj, b_proj):
    with_bias = bool(np.any(b_attn) or np.any(b_proj))
    nc = _get_nc(with_bias=with_bias)
    in_maps = make_in_maps(np.asarray(x, np.float32), np.asarray(w_attn, np.float32),
                           np.asarray(b_attn, np.float32),
                           np.asarray(w_proj, np.float32),
                           np.asarray(b_proj, np.float32))
    res = run_bass_kernel_spmd(nc, in_maps, core_ids=list(range(NCORES)))
    return gather_out(res.results)
